# revision 1
# baseline (speedup 1.0000x reference)
"""Trainium2 Bass kernel for nn_Attention_9122510537215 (gnn_message_passing).

Math (per batch b):
    Q = query @ Wq.T + bq                  [LQ=256, 256]
    K = input @ Wk.T + bk                  [LK, 256]
    V = input @ Wv.T + bv                  [LK, 256]
    alpha = softmax_k(Q @ K.T / 16)        [256, LK]
    out[j] = sum_k alpha[j, k] * V[k, j]   [256]

Algebraic restructuring used here:
  * bk shifts every score column by a constant along k -> cancels in softmax_k.
  * G[b] = Wk.T @ (query_b @ Wq.T + bq).T / 16, so scoresT = input @ G  ([LK, 256]).
  * Instead of materializing V, accumulate H[j, i] = sum_k e[k, j] * input[k, i]
    (e = exp(scores)); then numer[j] = sum_i H[j, i] * Wv[j, i] and an appended
    ones-column of the input yields denom[j] = H[j, 256].  bv is applied at the
    end:  out = numer / denom + bv.
  * Softmax is computed unnormalized without max-subtraction (scores are O(1)
    for this problem family; exp stays within a safe range).

Distribution: the LK (node) axis is zero-padded to 50176 = 8 * 6272 and sharded
across the 8 NeuronCores; each core returns its partial H accumulators
([B, 2, 128, 257] fp32) and the host reduces across cores in float64.
Padded rows carry a 0 in the ones-column so they contribute nothing.

Device layout: the host pre-casts the input to fp16 and ships both layouts so
the device does no transposes or casts:
  * "xn": natural rows, tile-transposed as [B, 128(part), 49(subtile), 258]
    so each partition reads one long contiguous run per DMA (>=4KB
    descriptors — descriptor rate, not bytes, limits the DMA engines).
    Node identity: k = subtile*128 + partition.
  * "xt": transposed [B, 256(i), 6272(k)] — k-contiguous per feature row.
TensorE streams fp16 matmuls (scores + H), ScalarE does exp, VectorE idle.
"""

import numpy as np
from contextlib import ExitStack

import concourse.bass as bass
import concourse.mybir as mybir
import concourse.tile as tile
from concourse import bacc
from concourse.bass_utils import run_bass_kernel_spmd

# Problem constants (hardcoded; kernel.py must be self-contained).
B = 4
LQ = 256
LK = 50000
OUT = 256
KV = 256            # input feature dim
NORM = 1.0 / 16.0   # 1/sqrt(OUT)

N_CORES = 8
SUB = 128                  # nodes per subtile (PE contraction width)
NSUB = 49                  # subtiles per core per batch
KS = NSUB * SUB            # 6272 nodes per core per batch
LK_PAD = KS * N_CORES      # 50176
CGRP = 4                   # subtiles per compute group (exp batching / PSUM)
DGRP = 8                   # subtiles per DMA group (descriptor batching)

F16 = mybir.dt.float16
F32 = mybir.dt.float32


def _round_robin(nc, names):
    state = [0]
    def pick():
        e = getattr(nc, names[state[0] % len(names)])
        state[0] += 1
        return e
    return pick


def build(ks=KS, repeat=1, dma_engines=("gpsimd", "sync", "scalar"),
          nat_bufs=3, tp_bufs=3, e_bufs=3, mode="full", dgrp=DGRP):
    """Emit the per-core SPMD Bass module (identical on all cores).

    repeat > 1 wraps the body in a hardware For_i loop recomputing the same
    result — used only for wall-clock benchmarking.
    mode: "full" (normal), "dma" (input loads only), "compute" (static inputs,
    no streaming loads) — ablation benchmarks.
    """
    nsub = ks // SUB
    cgroups = [CGRP] * (nsub // CGRP)
    if nsub % CGRP:
        cgroups.append(nsub % CGRP)

    nc = bacc.Bacc("TRN2", target_bir_lowering=False, debug=False,
                   num_devices=N_CORES)
    xn = nc.dram_tensor("xn", [B, 128, nsub, 258], F16, kind="ExternalInput")
    xt = nc.dram_tensor("xt", [B, 256, ks], F16, kind="ExternalInput")
    g = nc.dram_tensor("g", [B, 256, 256], F16, kind="ExternalInput")
    ht = nc.dram_tensor("ht", [B, 2, 128, 257], F32, kind="ExternalOutput")

    with ExitStack() as ctx:
        tc = ctx.enter_context(tile.TileContext(nc))
        gp = ctx.enter_context(tc.tile_pool(name="gp", bufs=1))
        natp = ctx.enter_context(tc.tile_pool(name="natp", bufs=nat_bufs))
        tpp = ctx.enter_context(tc.tile_pool(name="tpp", bufs=tp_bufs))
        ep = ctx.enter_context(tc.tile_pool(name="ep", bufs=e_bufs))
        hout = ctx.enter_context(tc.tile_pool(name="hout", bufs=2))
        spp = ctx.enter_context(tc.tile_pool(name="spp", bufs=2, space="PSUM"))
        hpp = ctx.enter_context(tc.tile_pool(name="hpp", bufs=2, space="PSUM"))

        # G for all batches, resident in SBUF: [i(2x128 part), q(256)].
        g_sb = gp.tile([128, B, 2, 256], F16)
        for b in range(B):
            for ih in range(2):
                nc.sync.dma_start(out=g_sb[:, b, ih, :],
                                  in_=g[b, ih * 128:(ih + 1) * 128, :])

        static_tiles = None
        e_static = None
        if mode in ("compute", "mmonly"):
            stp = ctx.enter_context(tc.tile_pool(name="static", bufs=1))
            snat = stp.tile([128, dgrp, 258], F16, tag="snat")
            nc.sync.dma_start(out=snat[:, :, :], in_=xn[0, :, 0:dgrp, :])
            stps = []
            for ih in range(2):
                t_ = stp.tile([128, dgrp * SUB], F16, tag=f"stp{ih}")
                nc.sync.dma_start(out=t_[:, :],
                                  in_=xt[0, ih * 128:(ih + 1) * 128, 0:dgrp * SUB])
                stps.append(t_)
            static_tiles = (snat, stps[0], stps[1])
            if mode == "mmonly":
                e_static = stp.tile([128, CGRP, 256], F16, tag="se")
                nc.vector.memset(e_static[:, :, :], 1.0)

        if mode.startswith("mm") and mode != "mmonly":
            # Pure PE microbenchmark: same MM count as the real kernel (784),
            # parameterized moving width N, all-static operands.
            # mm256/mm512/mm128: one stationary reused.
            # mmfresh: rotate 8 stationaries as strided slices of one buffer.
            # mmfresh2: rotate 8 dense stationary tiles.
            N = {"mm512": 512, "mm128": 128}.get(mode, 256)
            stp2 = ctx.enter_context(tc.tile_pool(name="static2", bufs=1))
            if mode in ("mmfresh", "mmpair", "mmht"):
                wbuf = stp2.tile([128, 8, 128], F16, tag="w")
                nc.vector.memset(wbuf[:, :, :], 0.5)
                ws = [wbuf[:, j, :] for j in range(8)]
            elif mode == "mmfresh2":
                ws = []
                for j in range(8):
                    wtile = stp2.tile([128, 128], F16, tag=f"w{j}")
                    nc.vector.memset(wtile[:, :], 0.5)
                    ws.append(wtile[:, :])
            else:
                w_static = stp2.tile([128, 128], F16, tag="w")
                nc.vector.memset(w_static[:, :], 0.5)
                ws = [w_static[:, :]] * 8
            r_static = stp2.tile([128, N], F16, tag="r")
            nc.vector.memset(r_static[:, :], 0.5)
            r258 = stp2.tile([128, 8, 258], F16, tag="r258")
            nc.vector.memset(r258[:, :, :], 0.5)
            spp2 = ctx.enter_context(
                tc.tile_pool(name="psum2", bufs=4, space="PSUM"))
            hpp2 = ctx.enter_context(
                tc.tile_pool(name="hpsum2", bufs=2, space="PSUM"))
            rep_ctx = tc.For_i(0, repeat, 1) if repeat > 1 else None
            if rep_ctx is not None:
                ctx.enter_context(rep_ctx)
            if mode == "mmpair":
                # scores-like: 2-MM accumulation pairs into rotating half-banks
                for grp in range(98):
                    o = spp2.tile([128, 4, 256], F32, tag="o4")
                    for i in range(4):
                        nc.tensor.matmul(o[:, i, :], ws[2 * i], r_static[:, :],
                                         start=True, stop=False)
                        nc.tensor.matmul(o[:, i, :], ws[2 * i + 1],
                                         r_static[:, :],
                                         start=False, stop=True)
            elif mode == "mmht":
                # HT-like: two persistent accumulators, N=257, alternating
                for rep_b in range(4):
                    a0 = hpp2.tile([128, 257], F32, tag="a0")
                    a1 = hpp2.tile([128, 257], F32, tag="a1")
                    for grp in range(49):
                        for i in range(2):
                            first = grp == 0 and i == 0
                            last = grp == 48 and i == 1
                            nc.tensor.matmul(a0[:, :], ws[2 * i],
                                             r258[:, 2 * i, 0:257],
                                             start=first, stop=last)
                            nc.tensor.matmul(a1[:, :], ws[2 * i + 1],
                                             r258[:, 2 * i + 1, 0:257],
                                             start=first, stop=last)
            else:
                for grp in range(98):
                    o = spp2.tile([128, N], F32, tag="o")
                    for j in range(8):
                        nc.tensor.matmul(o[:, :], ws[j], r_static[:, :],
                                         start=(j == 0), stop=(j == 7))
        else:
            rep_ctx = tc.For_i(0, repeat, 1) if repeat > 1 else None
            if rep_ctx is not None:
                ctx.enter_context(rep_ctx)

        n_cg = len(cgroups)
        pick_eng = _round_robin(nc, dma_engines)
        n_batches = 0 if (mode.startswith("mm") and mode != "mmonly") else B
        for b in range(n_batches):
            ht0 = hpp.tile([128, 257], F32, tag="ht0")
            ht1 = hpp.tile([128, 257], F32, tag="ht1")
            dma_tiles = {}  # dma-group index -> (nat, tp0, tp1)

            def load_dgroup(d):
                if mode == "compute":
                    dma_tiles[d] = static_tiles
                    return
                d0 = d * dgrp
                dsz = min(dgrp, nsub - d0)
                natt = natp.tile([128, dgrp, 258], F16, tag="nat")
                pick_eng().dma_start(out=natt[:, :dsz, :],
                                     in_=xn[b, :, d0:d0 + dsz, :])
                tps = []
                for ih in range(2):
                    tptt = tpp.tile([128, dgrp * SUB], F16, tag=f"tp{ih}")
                    pick_eng().dma_start(
                        out=tptt[:, :dsz * SUB],
                        in_=xt[b, ih * 128:(ih + 1) * 128,
                               d0 * SUB:(d0 + dsz) * SUB])
                    tps.append(tptt)
                dma_tiles[d] = (natt, tps[0], tps[1])

            pend = None  # (e, subtile_list, is_first) of previous compute group
            for t in range(n_cg + 1):
                if t < n_cg:
                    sz = cgroups[t]
                    subs = [t * CGRP + i for i in range(sz)]
                    for s in subs:
                        if s // dgrp not in dma_tiles:
                            load_dgroup(s // dgrp)
                    if mode == "dma":
                        continue
                    # scoresT[k, q] = sum_i inpT[i, k].T @ G[i, q]
                    sp = spp.tile([128, CGRP, 256], F32)
                    for i, s in enumerate(subs):
                        natt, tp0, tp1 = dma_tiles[s // dgrp]
                        r = s % dgrp
                        nc.tensor.matmul(sp[:, i, :],
                                         tp0[:, r * SUB:(r + 1) * SUB],
                                         g_sb[:, b, 0, :],
                                         start=True, stop=False)
                        nc.tensor.matmul(sp[:, i, :],
                                         tp1[:, r * SUB:(r + 1) * SUB],
                                         g_sb[:, b, 1, :],
                                         start=False, stop=True)
                    if mode == "mmonly":
                        e = e_static
                    else:
                        e = ep.tile([128, CGRP, 256], F16)
                        nc.scalar.activation(e[:, :sz, :], sp[:, :sz, :],
                                             mybir.ActivationFunctionType.Exp)
                # H matmuls of the previous compute group (keeps PE busy while
                # ScalarE computes this group's exp).
                if mode == "scores":
                    pend = None
                if pend is not None:
                    pe_, psubs, pfirst = pend
                    for i, s in enumerate(psubs):
                        natt = dma_tiles[s // dgrp][0]
                        is_first = pfirst and i == 0
                        is_last = (t == n_cg) and i == len(psubs) - 1
                        nc.tensor.matmul(ht0[:, :], pe_[:, i, 0:128],
                                         natt[:, s % dgrp, 0:257],
                                         start=is_first, stop=is_last)
                        nc.tensor.matmul(ht1[:, :], pe_[:, i, 128:256],
                                         natt[:, s % dgrp, 0:257],
                                         start=is_first, stop=is_last)
                if t < n_cg:
                    pend = (e, subs, t == 0)
            if mode in ("dma", "scores"):
                continue
            hts = hout.tile([128, 2, 257], F32)
            nc.vector.tensor_copy(hts[:, 0, :], ht0[:, :])
            nc.vector.tensor_copy(hts[:, 1, :], ht1[:, :])
            nc.sync.dma_start(out=ht[b, 0], in_=hts[:, 0, :])
            nc.sync.dma_start(out=ht[b, 1], in_=hts[:, 1, :])
    nc.compile()
    return nc


def _prepare_inputs(query, input, Wq, bq, Wk):
    """Host-side marshalling: G matrices + fp16 input in both layouts, sharded."""
    # G[b] = Wk.T @ (query_b @ Wq.T + bq).T * NORM   -> [B, 256(i), 256(q)]
    Q = query.astype(np.float64) @ Wq.T.astype(np.float64) + bq
    G = np.einsum('di,bqd->biq', Wk.astype(np.float64), Q) * NORM
    g16 = np.ascontiguousarray(G.astype(np.float32).astype(np.float16))

    xn = np.zeros((B, LK_PAD, 258), np.float16)
    xn[:, :LK, :256] = input.astype(np.float16)
    xn[:, :LK, 256] = 1.0   # ones-column -> denom; stays 0 on padded rows
    xt_view = xn[:, :, :256].transpose(0, 2, 1)  # [B, 256, LK_PAD] view

    in_maps = []
    for c in range(N_CORES):
        sl = slice(c * KS, (c + 1) * KS)
        # natural, tile-transposed: [B, 128, NSUB, 258]; node k = t*128 + p
        xn_c = xn[:, sl, :].reshape(B, NSUB, 128, 258).transpose(0, 2, 1, 3)
        in_maps.append({
            "xn": np.ascontiguousarray(xn_c),
            "xt": np.ascontiguousarray(xt_view[:, :, sl]),
            "g": g16,
        })
    return in_maps


def kernel(query, input, Wq, bq, Wk, bk, Wv, bv):
    # bk provably cancels in softmax over k; bq is folded into G; bv is applied
    # in the host-side epilogue below.
    query = np.asarray(query, dtype=np.float32)
    input = np.asarray(input, dtype=np.float32)
    Wq = np.asarray(Wq, dtype=np.float32)
    bq = np.asarray(bq, dtype=np.float32)
    Wk = np.asarray(Wk, dtype=np.float32)
    Wv = np.asarray(Wv, dtype=np.float32)
    bv = np.asarray(bv, dtype=np.float32)

    nc = build()
    in_maps = _prepare_inputs(query, input, Wq, bq, Wk)
    res = run_bass_kernel_spmd(nc, in_maps, core_ids=list(range(N_CORES)))
    kernel._last_result = res

    numer = np.zeros((B, OUT))
    denom = np.zeros((B, OUT))
    Wv64 = Wv.astype(np.float64)
    for r in res.results:
        H = r["ht"].astype(np.float64).reshape(B, OUT, 257)  # j = half*128 + p
        numer += (H[:, :, :256] * Wv64[None]).sum(axis=2)
        denom += H[:, :, 256]
    out = numer / denom + bv
    return out.astype(np.float32)


if __name__ == "__main__":
    # CoreSim smoke test on a reduced size (5 subtiles -> cgroups [4, 1]).
    from concourse.bass_interp import CoreSim

    nsub_t = 5
    ks = nsub_t * SUB
    rng = np.random.default_rng(0)
    xn_np = rng.standard_normal((B, ks, 258)).astype(np.float16)
    xn_np[:, :, 256] = 1.0
    xn_np[:, :, 257] = 0.0
    xt_np = np.ascontiguousarray(xn_np[:, :, :256].transpose(0, 2, 1))
    xn_tiled = np.ascontiguousarray(
        xn_np.reshape(B, nsub_t, 128, 258).transpose(0, 2, 1, 3))
    g_np = (rng.standard_normal((B, 256, 256)) * 0.05).astype(np.float16)

    nc = build(ks=ks)
    sim = CoreSim(nc)
    sim.tensor("xn")[:] = xn_tiled
    sim.tensor("xt")[:] = xt_np
    sim.tensor("g")[:] = g_np
    sim.simulate()
    got = np.array(sim.tensor("ht")).reshape(B, OUT, 257)

    x = xn_np[:, :, :257].astype(np.float32)
    want = np.zeros((B, OUT, 257), np.float32)
    for b in range(B):
        s = x[b, :, :256] @ g_np[b].astype(np.float32)
        e = np.exp(s).astype(np.float16).astype(np.float32)
        want[b] = e.T @ x[b]
    err = np.abs(got - want).max() / np.abs(want).max()
    print("CoreSim rel err:", err)
    assert err < 2e-2, err
    print("OK")



# revision 5
# speedup vs baseline: 1.0014x; 1.0014x over previous
"""Trainium2 Bass kernel for nn_Attention_9122510537215 (gnn_message_passing).

Math (per batch b):
    Q = query @ Wq.T + bq                  [LQ=256, 256]
    K = input @ Wk.T + bk                  [LK, 256]
    V = input @ Wv.T + bv                  [LK, 256]
    alpha = softmax_k(Q @ K.T / 16)        [256, LK]
    out[j] = sum_k alpha[j, k] * V[k, j]   [256]

Restructure vs the two-layout baseline:
  * bk shifts every score column by a constant along k -> cancels in softmax_k.
  * G[b] = Wk.T @ (query_b @ Wq.T + bq).T / 16, so s[q, k] = (G.T @ x.T)[q, k].
  * vT[j, k] = (Wv @ x.T)[j, k] is computed ON DEVICE from the SAME moving
    operand as the scores (x.T), with Wv.T stationary.  Then
        numer[j] = sum_k e[j, k] * vT[j, k],   denom[j] = sum_k e[j, k]
    and out = numer / denom + bv (bv applied on host; scores are O(1) so the
    softmax runs unnormalized without max-subtraction).
  * Only ONE layout of the input is shipped (x.T, features-on-partitions):
    half the HBM traffic of the baseline.  G and Wv.T are the only PE
    stationaries (8 loads per 512-column chunk, fully hidden), the moving
    stream is x.T.
  * e = exp(s) runs on ScalarE with a fused free-axis accumulate (denom for
    free).  numer's multiply+reduce is split between VectorE
    (tensor_tensor_reduce) and GpSimd (scalar_tensor_tensor) so no single
    engine bottlenecks.  G / Wv are pre-scaled by 16 on the host (exp applies
    scale=1/16; numer is divided by 16 on the host).

Distribution: the LK (node) axis is zero-padded to 50176 = 8 * 6272 and
sharded across the 8 NeuronCores; each core returns per-chunk column sums
[B, 128, 2(half), 2(numer/denom), NCOL] fp32 and the host reduces in float64.
Padded rows have x = 0 -> s = 0 -> e = 1 exactly, contributing 0 to numer and
+176 (total, last core only) to denom: subtracted exactly on the host.
"""

import numpy as np
from contextlib import ExitStack

import ml_dtypes

import concourse.bass as bass
import concourse.mybir as mybir
import concourse.tile as tile
from concourse import bacc
from concourse.bass_utils import run_bass_kernel_spmd

# Problem constants (hardcoded; kernel.py must be self-contained).
B = 4
LQ = 256
LK = 50000
OUT = 256
KV = 256            # input feature dim
NORM = 1.0 / 16.0   # 1/sqrt(OUT)
PRESCALE = 16.0     # host multiplies G and Wv by this; undone on device/host

N_CORES = 8
KS = 6272                  # nodes per core per batch (49 * 128)
LK_PAD = KS * N_CORES      # 50176
N_PAD = LK_PAD - LK        # 176 zero rows, all on the last core
CHUNK = 512                # moving columns per PSUM bank

F16 = mybir.dt.float16
F32 = mybir.dt.float32
F8 = mybir.dt.float8e4

ALU = mybir.AluOpType
AF = mybir.ActivationFunctionType


def _chunks(ks):
    out = []
    c0 = 0
    while c0 < ks:
        out.append((c0, min(CHUNK, ks - c0)))
        c0 += CHUNK
    return out


def build(ks=KS, fp8=False):
    """Emit the per-core SPMD Bass module (identical on all cores).

    fp8: x / G / Wv are fp8e4 and the four matmuls per chunk run in DoubleRow
    mode (contraction 256 in one pass).  Otherwise fp16.
    """
    chunks = _chunks(ks)
    ncol = len(chunks)
    DT = F8 if fp8 else F16

    nc = bacc.Bacc("TRN2", target_bir_lowering=False, debug=False,
                   num_devices=N_CORES)
    if fp8:
        # DoubleRow operand layouts: [partition p, slot o, cols]; contraction
        # index i = o * 128 + p.
        xt = nc.dram_tensor("xt", [B, 128, 2, ks], DT, kind="ExternalInput")
        g = nc.dram_tensor("g", [B, 128, 2, 256], DT, kind="ExternalInput")
        wv = nc.dram_tensor("wv", [128, 2, 256], DT, kind="ExternalInput")
    else:
        # [b, i-half, i-partition, cols]
        xt = nc.dram_tensor("xt", [B, 2, 128, ks], DT, kind="ExternalInput")
        g = nc.dram_tensor("g", [B, 2, 128, 256], DT, kind="ExternalInput")
        wv = nc.dram_tensor("wv", [2, 128, 256], DT, kind="ExternalInput")
    oc = nc.dram_tensor("oc", [B, 128, 2, 2, ncol], F32, kind="ExternalOutput")

    with ExitStack() as ctx:
        tc = ctx.enter_context(tile.TileContext(nc))
        wp = ctx.enter_context(tc.tile_pool(name="wp", bufs=1))
        xp = ctx.enter_context(tc.tile_pool(name="xp", bufs=1))
        pp = ctx.enter_context(tc.tile_pool(name="pp", bufs=2, space="PSUM"))
        ep = ctx.enter_context(tc.tile_pool(name="ep", bufs=3))
        sp = ctx.enter_context(tc.tile_pool(name="sp", bufs=2))
        ocp = ctx.enter_context(tc.tile_pool(name="ocp", bufs=2))

        if fp8:
            g_sb = wp.tile([128, B, 2, 256], DT, tag="g")
            wv_sb = wp.tile([128, 2, 256], DT, tag="wv")
            x_sb = xp.tile([128, B, 2, ks], DT, tag="x")
            for b in range(B):
                nc.scalar.dma_start(out=g_sb[:, b], in_=g[b])
            nc.scalar.dma_start(out=wv_sb[:, :, :], in_=wv[:, :, :])
            for b in range(B):
                nc.sync.dma_start(out=x_sb[:, b], in_=xt[b])

            def mm_s(out_ap, b, qh, c0, csz):
                nc.tensor.matmul(
                    out_ap,
                    g_sb[:, b, :, qh * 128:(qh + 1) * 128],
                    x_sb[:, b, :, c0:c0 + csz],
                    start=True, stop=True,
                    perf_mode=mybir.MatmulPerfMode.DoubleRow)

            def mm_v(out_ap, b, jh, c0, csz):
                nc.tensor.matmul(
                    out_ap,
                    wv_sb[:, :, jh * 128:(jh + 1) * 128],
                    x_sb[:, b, :, c0:c0 + csz],
                    start=True, stop=True,
                    perf_mode=mybir.MatmulPerfMode.DoubleRow)
        else:
            g_sb = wp.tile([128, B, 2, 256], DT, tag="g")
            wv_sb = wp.tile([128, 2, 256], DT, tag="wv")
            x_sb = xp.tile([128, B, 2, ks], DT, tag="x")
            for b in range(B):
                for ih in range(2):
                    nc.scalar.dma_start(out=g_sb[:, b, ih], in_=g[b, ih])
            for ih in range(2):
                nc.scalar.dma_start(out=wv_sb[:, ih], in_=wv[ih])
            for b in range(B):
                for ih in range(2):
                    nc.sync.dma_start(out=x_sb[:, b, ih], in_=xt[b, ih])

            def mm_s(out_ap, b, qh, c0, csz):
                for ih in range(2):
                    nc.tensor.matmul(
                        out_ap,
                        g_sb[:, b, ih, qh * 128:(qh + 1) * 128],
                        x_sb[:, b, ih, c0:c0 + csz],
                        start=(ih == 0), stop=(ih == 1))

            def mm_v(out_ap, b, jh, c0, csz):
                for ih in range(2):
                    nc.tensor.matmul(
                        out_ap,
                        wv_sb[:, ih, jh * 128:(jh + 1) * 128],
                        x_sb[:, b, ih, c0:c0 + csz],
                        start=(ih == 0), stop=(ih == 1))

        for b in range(B):
            occ = ocp.tile([128, 2, 2, ncol], F32, tag="occ")
            for t, (c0, csz) in enumerate(chunks):
                s_t = [pp.tile([128, CHUNK], F32, tag=f"s{h}", name=f"s{h}")
                       for h in range(2)]
                v_t = [pp.tile([128, CHUNK], F32, tag=f"v{h}", name=f"v{h}")
                       for h in range(2)]
                for h in range(2):
                    mm_s(s_t[h][:, :csz], b, h, c0, csz)
                    mm_v(v_t[h][:, :csz], b, h, c0, csz)
                e_t = []
                for h in range(2):
                    e_ = ep.tile([128, CHUNK], F16, tag=f"e{h}")
                    nc.scalar.activation(
                        e_[:, :csz], s_t[h][:, :csz], AF.Exp,
                        scale=1.0 / PRESCALE,
                        accum_out=occ[:, h, 1, t:t + 1])
                    e_t.append(e_)
                # numer: fused multiply+sum on VectorE via the standard
                # TensorScalarPtr instruction (GpSimd cannot read PSUM on real
                # HW, and the custom-ucode tensor_tensor_reduce faults there)
                for h in range(2):
                    p_ = sp.tile([128, CHUNK], F16, tag=f"p{h}", name=f"p{h}")
                    nc.vector.scalar_tensor_tensor(
                        out=p_[:, :csz],
                        in0=v_t[h][:, :csz], scalar=1.0, in1=e_t[h][:, :csz],
                        op0=ALU.mult, op1=ALU.mult,
                        accum_out=occ[:, h, 0, t:t + 1])
            nc.sync.dma_start(out=oc[b], in_=occ[:, :, :, :])
    nc.compile()
    return nc


def _to_fp8(a):
    return np.clip(a, -240.0, 240.0).astype(ml_dtypes.float8_e4m3)


def _prepare_inputs(query, input, Wq, bq, Wk, Wv, fp8=False):
    """Host-side marshalling: G (incl. bq, 1/16, PRESCALE), Wv.T, x.T shards."""
    Q = query.astype(np.float64) @ Wq.T.astype(np.float64) + bq
    G = np.einsum('di,bqd->biq', Wk.astype(np.float64), Q) * (NORM * PRESCALE)
    WvT = Wv.T.astype(np.float64) * PRESCALE           # [i, j]

    xpad = np.zeros((B, LK_PAD, KV), np.float32)
    xpad[:, :LK] = input
    xT = xpad.transpose(0, 2, 1)                       # [B, 256, LK_PAD] view

    if fp8:
        g8 = _to_fp8(G.reshape(B, 2, 128, 256).transpose(0, 2, 1, 3))
        wv8 = _to_fp8(WvT.reshape(2, 128, 256).transpose(1, 0, 2))
        g8 = np.ascontiguousarray(g8)
        wv8 = np.ascontiguousarray(wv8)
        in_maps = []
        for c in range(N_CORES):
            sl = slice(c * KS, (c + 1) * KS)
            xc = xT[:, :, sl].reshape(B, 2, 128, KS).transpose(0, 2, 1, 3)
            in_maps.append({"xt": np.ascontiguousarray(_to_fp8(xc)),
                            "g": g8, "wv": wv8})
    else:
        g16 = np.ascontiguousarray(
            G.astype(np.float32).astype(np.float16).reshape(B, 2, 128, 256))
        wv16 = np.ascontiguousarray(
            WvT.astype(np.float32).astype(np.float16).reshape(2, 128, 256))
        in_maps = []
        for c in range(N_CORES):
            sl = slice(c * KS, (c + 1) * KS)
            xc = xT[:, :, sl].reshape(B, 2, 128, KS)
            in_maps.append({"xt": np.ascontiguousarray(xc.astype(np.float16)),
                            "g": g16, "wv": wv16})
    return in_maps


USE_FP8 = False


def kernel(query, input, Wq, bq, Wk, bk, Wv, bv):
    # bk provably cancels in softmax over k; bq is folded into G; bv is applied
    # in the host-side epilogue below.
    query = np.asarray(query, dtype=np.float32)
    input = np.asarray(input, dtype=np.float32)
    Wq = np.asarray(Wq, dtype=np.float32)
    bq = np.asarray(bq, dtype=np.float32)
    Wk = np.asarray(Wk, dtype=np.float32)
    Wv = np.asarray(Wv, dtype=np.float32)
    bv = np.asarray(bv, dtype=np.float32)

    nc = build(fp8=USE_FP8)
    in_maps = _prepare_inputs(query, input, Wq, bq, Wk, Wv, fp8=USE_FP8)
    res = run_bass_kernel_spmd(nc, in_maps, core_ids=list(range(N_CORES)))
    kernel._last_result = res

    numer = np.zeros((B, 2, 128))
    denom = np.zeros((B, 2, 128))
    for r in res.results:
        o = r["oc"].astype(np.float64)       # [B, 128, 2, 2, ncol]
        numer += o[:, :, :, 0, :].sum(axis=3).transpose(0, 2, 1)
        denom += o[:, :, :, 1, :].sum(axis=3).transpose(0, 2, 1)
    numer = numer.reshape(B, OUT) / PRESCALE
    denom = denom.reshape(B, OUT) - N_PAD    # padded rows contribute e=1 each
    out = numer / denom + bv
    return out.astype(np.float32)


if __name__ == "__main__":
    # CoreSim smoke test on a reduced size (2.25 chunks -> [512, 512, 128]).
    from concourse.bass_interp import CoreSim

    for fp8 in (False, True):
        ks = 1152
        rng = np.random.default_rng(0)
        x = rng.standard_normal((B, ks, KV)).astype(np.float32)
        G = (rng.standard_normal((B, KV, 256)) * 0.4).astype(np.float64)
        WvT = (rng.standard_normal((KV, 256)) * 0.8).astype(np.float64)

        nc = build(ks=ks, fp8=fp8)
        sim = CoreSim(nc)
        xT = x.transpose(0, 2, 1)  # [B, 256, ks]
        if fp8:
            sim.tensor("xt")[:] = _to_fp8(
                xT.reshape(B, 2, 128, ks).transpose(0, 2, 1, 3))
            sim.tensor("g")[:] = _to_fp8(
                G.reshape(B, 2, 128, 256).transpose(0, 2, 1, 3))
            sim.tensor("wv")[:] = _to_fp8(
                WvT.reshape(2, 128, 256).transpose(1, 0, 2))
            xq = _to_fp8(xT).astype(np.float64)
            gq = _to_fp8(G).astype(np.float64)
            wq = _to_fp8(WvT).astype(np.float64)
        else:
            sim.tensor("xt")[:] = xT.reshape(B, 2, 128, ks).astype(np.float16)
            sim.tensor("g")[:] = G.astype(np.float16).reshape(B, 2, 128, 256)
            sim.tensor("wv")[:] = WvT.astype(np.float16).reshape(2, 128, 256)
            xq = xT.astype(np.float16).astype(np.float64)
            gq = G.astype(np.float16).astype(np.float64)
            wq = WvT.astype(np.float16).astype(np.float64)
        sim.simulate()
        got = np.array(sim.tensor("oc")).astype(np.float64)  # [B,128,2,2,ncol]
        gnum = got[:, :, :, 0, :].sum(axis=3).transpose(0, 2, 1).reshape(B, 256)
        gden = got[:, :, :, 1, :].sum(axis=3).transpose(0, 2, 1).reshape(B, 256)

        wnum = np.zeros((B, 256))
        wden = np.zeros((B, 256))
        for b in range(B):
            s = (gq[b].T @ xq[b]) / PRESCALE          # [256 q, ks]
            e = np.exp(s)
            v = wq.T @ xq[b]                          # [256 j, ks]
            e16 = e.astype(np.float16).astype(np.float64)
            wnum[b] = (e16 * v).sum(axis=1)
            wden[b] = e16.sum(axis=1)
        en = np.abs(gnum - wnum).max() / np.abs(wnum).max()
        ed = np.abs(gden - wden).max() / np.abs(wden).max()
        print(f"fp8={fp8}: CoreSim numer rel err {en:.3e}, denom rel err {ed:.3e}")
        assert en < 2e-2 and ed < 2e-2, (en, ed)
    print("OK")
